# revision 1
# baseline (speedup 1.0000x reference)
"""Trainium2 Bass kernel for 1D multi-scale deformable attention.

Self-contained: builds the Bass/Tile program, shards the full inputs
data-parallel over N across 8 NeuronCores, runs via run_bass_kernel_spmd,
and returns the full (N, LQ, 256) output.

Algorithm per core (one batch element):
  value = vin @ W_val.T + b_val            -> padded natural layout (T', 256)
  offs' = q @ (W_off/T).T + b_off/T        -> x = ref + offs' ; ix = x*T - 0.5
  attn  = softmax(q @ W_attn.T + b_attn)   per (q, m) over 16 (l,p)
  bilinear + zero padding == sum_t relu(1 - |ix - t|) * V[t], t in [0, T)
  per (q,l): one all-head window, base = min over (m,p) of needlo (clamped),
  static width W_l; indirect-DMA gathers W_l full 1KB value rows per query
  u[m,j] = sum_p attn * relu(1 - |ix_p - (base+j)|)
  out[q, m*32+d] = sum_{l,j} u * G
"""
import os
import numpy as np
from contextlib import ExitStack

import concourse.bass as bass
import concourse.bacc as bacc
import concourse.tile as tile
from concourse import mybir
from concourse.masks import make_identity
from concourse.bass_utils import run_bass_kernel_spmd

f32 = mybir.dt.float32
i32 = mybir.dt.int32
ALU = mybir.AluOpType
ACT = mybir.ActivationFunctionType

# static problem config
LENS = (2048, 1024, 512, 256)
N, LQ, DM = 8, 2048, 256
M, L, P, DH = 8, 4, 4, 32
S = sum(LENS)                      # 3840
WCONF = (8, 10, 8, 10)             # per-level all-head window rows (l3 padded to pair l1)
PAD = 12                           # zero rows after each level (>= max(W)-1)
LSTARTP = []
_s = 0
for _T in LENS:
    LSTARTP.append(_s)
    _s += _T + PAD
TPR = _s                           # 3888 padded rows total
NQT = LQ // 128                    # 16 query tiles
NVT = S // 128                     # 30 value tiles
BIG = 100000.0

# consts layout (one row, broadcast to 128 partitions at load)
C_TVEC = 0           # 128: T_l per c (c = m*16+l*4+p)
C_TM1L = 128         # 4:  T_l - 1
C_LST = 132          # 4:  LSTARTP[l]
C_JROW = 136         # 16: j = 0..15
C_NEG1 = 152         # 1: -1.0
CW = 153


def _ap(base, dims, extra_offset=0):
    """Custom strided AP derived from a 2D (128, F) contiguous tile AP."""
    return bass.AP(
        tensor=base.tensor,
        offset=base.offset + extra_offset,
        ap=[list(base.ap[0])] + [[s, c] for s, c in dims],
    )


def build_program():
    nc = bacc.Bacc("TRN2", target_bir_lowering=False, debug=False)

    q_d = nc.dram_tensor("q", [LQ, DM], f32, kind="ExternalInput")
    ref_d = nc.dram_tensor("ref", [LQ, L], f32, kind="ExternalInput")
    vin_d = nc.dram_tensor("vin", [S, DM], f32, kind="ExternalInput")
    wv_d = nc.dram_tensor("wv", [DM + 1, DM], f32, kind="ExternalInput")
    wof_d = nc.dram_tensor("wof", [DM + 1, M * L * P], f32, kind="ExternalInput")
    wat_d = nc.dram_tensor("wat", [DM + 1, M * L * P], f32, kind="ExternalInput")
    consts_d = nc.dram_tensor("consts", [1, CW], f32, kind="ExternalInput")
    out_d = nc.dram_tensor("out", [LQ, DM], f32, kind="ExternalOutput")

    with tile.TileContext(nc) as tc, ExitStack() as ctx:
        singles = ctx.enter_context(tc.tile_pool(name="singles", bufs=1))
        dram = ctx.enter_context(tc.tile_pool(name="dram", bufs=1, space="DRAM"))
        vpool = ctx.enter_context(tc.tile_pool(name="vpool", bufs=3))
        psum = ctx.enter_context(tc.tile_pool(name="psum", bufs=2, space="PSUM"))
        qpool = ctx.enter_context(tc.tile_pool(name="qpool", bufs=2))
        gpool = ctx.enter_context(tc.tile_pool(name="gpool", bufs=2))
        spool = ctx.enter_context(tc.tile_pool(name="spool", bufs=2))

        # ---- constants / weights (loaded once)
        ident = singles.tile([128, 128], f32)
        make_identity(nc, ident[:])
        ones_row = singles.tile([1, 128], f32)
        nc.vector.memset(ones_row[:], 1.0)
        consts = singles.tile([128, CW], f32)
        nc.sync.dma_start(
            out=consts[:],
            in_=bass.AP(tensor=consts_d[:].tensor, offset=0,
                        ap=[[0, 128], [1, CW]]),
        )
        wv0 = singles.tile([128, DM], f32)
        wv1 = singles.tile([128, DM], f32)
        wvb = singles.tile([1, DM], f32)
        nc.sync.dma_start(out=wv0[:], in_=wv_d[0:128, :])
        nc.sync.dma_start(out=wv1[:], in_=wv_d[128:256, :])
        nc.sync.dma_start(out=wvb[:], in_=wv_d[256:257, :])
        wof0 = singles.tile([128, 128], f32)
        wof1 = singles.tile([128, 128], f32)
        wofb = singles.tile([1, 128], f32)
        nc.sync.dma_start(out=wof0[:], in_=wof_d[0:128, :])
        nc.sync.dma_start(out=wof1[:], in_=wof_d[128:256, :])
        nc.sync.dma_start(out=wofb[:], in_=wof_d[256:257, :])
        wat0 = singles.tile([128, 128], f32)
        wat1 = singles.tile([128, 128], f32)
        watb = singles.tile([1, 128], f32)
        nc.sync.dma_start(out=wat0[:], in_=wat_d[0:128, :])
        nc.sync.dma_start(out=wat1[:], in_=wat_d[128:256, :])
        nc.sync.dma_start(out=watb[:], in_=wat_d[256:257, :])

        # ---- value scratch: natural padded rows (TPR, 256)
        vp = dram.tile([TPR, DM], f32)
        zt = singles.tile([128, DM], f32)
        nc.vector.memset(zt[:], 0.0)
        for l, T in enumerate(LENS):
            nc.sync.dma_start(
                out=vp[:][LSTARTP[l] + T:LSTARTP[l] + T + PAD, :],
                in_=zt[:PAD, :])

        # ---- phase A: value projection into vp
        for tt in range(NVT):
            vt = vpool.tile([128, DM], f32, tag="vt")
            nc.sync.dma_start(out=vt[:], in_=vin_d[tt * 128:(tt + 1) * 128, :])
            ps0 = psum.tile([128, 128], f32, tag="tr")
            ps1 = psum.tile([128, 128], f32, tag="tr")
            nc.tensor.transpose(out=ps0[:], in_=vt[:, 0:128], identity=ident[:])
            nc.tensor.transpose(out=ps1[:], in_=vt[:, 128:256], identity=ident[:])
            vT0 = vpool.tile([128, 128], f32, tag="vT")
            vT1 = vpool.tile([128, 128], f32, tag="vT")
            nc.scalar.copy(out=vT0[:], in_=ps0[:])
            nc.scalar.copy(out=vT1[:], in_=ps1[:])
            pv = psum.tile([128, DM], f32, tag="mm")
            nc.tensor.matmul(out=pv[:], lhsT=vT0[:], rhs=wv0[:], start=True, stop=False)
            nc.tensor.matmul(out=pv[:], lhsT=vT1[:], rhs=wv1[:], start=False, stop=False)
            nc.tensor.matmul(out=pv[:], lhsT=ones_row[:], rhs=wvb[:], start=False, stop=True)
            st = vpool.tile([128, DM], f32, tag="st")
            nc.scalar.copy(out=st[:], in_=pv[:])
            row0 = tt * 128
            acc = 0
            for li, T in enumerate(LENS):
                if row0 < acc + T:
                    l, trel = li, row0 - acc
                    break
                acc += T
            dst = LSTARTP[l] + trel
            nc.sync.dma_start(out=vp[:][dst:dst + 128, :], in_=st[:])

        # ---- phase B: per query tile (optionally unrolled repeats for timing)
        rep = max(1, int(os.environ.get("DEFORM_REPEAT", "1")))
        for qt in [i % NQT for i in range(rep * NQT)]:
            qtile = qpool.tile([128, DM], f32, tag="qtile")
            reft = qpool.tile([128, L], f32, tag="reft")
            nc.sync.dma_start(out=qtile[:], in_=q_d[qt * 128:(qt + 1) * 128, :])
            nc.sync.dma_start(out=reft[:], in_=ref_d[qt * 128:(qt + 1) * 128, :])

            psq0 = psum.tile([128, 128], f32, tag="tr")
            psq1 = psum.tile([128, 128], f32, tag="tr")
            nc.tensor.transpose(out=psq0[:], in_=qtile[:, 0:128], identity=ident[:])
            nc.tensor.transpose(out=psq1[:], in_=qtile[:, 128:256], identity=ident[:])
            qT0 = qpool.tile([128, 128], f32, tag="qT")
            qT1 = qpool.tile([128, 128], f32, tag="qT")
            nc.scalar.copy(out=qT0[:], in_=psq0[:])
            nc.scalar.copy(out=qT1[:], in_=psq1[:])

            offp = psum.tile([128, 128], f32, tag="mm")
            nc.tensor.matmul(out=offp[:], lhsT=qT0[:], rhs=wof0[:], start=True, stop=False)
            nc.tensor.matmul(out=offp[:], lhsT=qT1[:], rhs=wof1[:], start=False, stop=False)
            nc.tensor.matmul(out=offp[:], lhsT=ones_row[:], rhs=wofb[:], start=False, stop=True)
            attp = psum.tile([128, 128], f32, tag="mm")
            nc.tensor.matmul(out=attp[:], lhsT=qT0[:], rhs=wat0[:], start=True, stop=False)
            nc.tensor.matmul(out=attp[:], lhsT=qT1[:], rhs=wat1[:], start=False, stop=False)
            nc.tensor.matmul(out=attp[:], lhsT=ones_row[:], rhs=watb[:], start=False, stop=True)

            # softmax (no max-sub: |logits| < ~4)
            E = qpool.tile([128, 128], f32, tag="E")
            nc.scalar.activation(out=E[:], in_=attp[:], func=ACT.Exp)
            sm = qpool.tile([128, M], f32, tag="sm")
            nc.vector.tensor_reduce(out=sm[:], in_=E[:].rearrange("p (m k) -> p m k", m=M),
                                    axis=mybir.AxisListType.X, op=ALU.add)
            rr = qpool.tile([128, M], f32, tag="rr")
            nc.vector.reciprocal(out=rr[:], in_=sm[:])
            A = qpool.tile([128, 128], f32, tag="A")
            nc.vector.tensor_tensor(out=A[:], in0=E[:],
                                    in1=_ap(rr[:], [[1, M], [0, 16]]), op=ALU.mult)

            # ix = (ref + offs/T)*T - 0.5
            X = qpool.tile([128, 128], f32, tag="X")
            nc.vector.tensor_tensor(out=X[:], in0=offp[:],
                                    in1=_ap(reft[:], [[0, M], [1, L], [0, P]]),
                                    op=ALU.add)
            IX = qpool.tile([128, 128], f32, tag="IX")
            nc.vector.tensor_tensor(out=IX[:], in0=X[:],
                                    in1=consts[:, C_TVEC:C_TVEC + 128], op=ALU.mult)
            nc.vector.tensor_scalar(out=IX[:], in0=IX[:], scalar1=0.5, scalar2=None,
                                    op0=ALU.subtract)

            # needlo per point: relu -> floor (int cast) ; dead mask via relu
            REL = qpool.tile([128, 128], f32, tag="REL")
            nc.vector.tensor_scalar(out=REL[:], in0=IX[:], scalar1=0.0, scalar2=None,
                                    op0=ALU.max)
            FLI = qpool.tile([128, 128], i32, tag="FLI")
            nc.vector.tensor_copy(out=FLI[:], in_=REL[:])
            FLR = qpool.tile([128, 128], f32, tag="FLR")
            nc.vector.tensor_copy(out=FLR[:], in_=FLI[:])
            GT = qpool.tile([128, 128], f32, tag="GT")
            nc.vector.tensor_tensor(out=GT[:], in0=FLR[:], in1=REL[:], op=ALU.is_gt)
            FL = qpool.tile([128, 128], f32, tag="FL")
            nc.vector.tensor_tensor(out=FL[:], in0=FLR[:], in1=GT[:], op=ALU.subtract)
            MSK = qpool.tile([128, 128], f32, tag="MSK")
            nc.scalar.activation(out=MSK[:], in_=IX[:], func=ACT.Relu,
                                 bias=consts[:, C_NEG1:C_NEG1 + 1], scale=-1.0)
            nc.vector.tensor_scalar(out=MSK[:], in0=MSK[:], scalar1=1e13,
                                    scalar2=BIG, op0=ALU.mult, op1=ALU.min)
            NL = qpool.tile([128, 128], f32, tag="NL")
            nc.vector.tensor_tensor(out=NL[:], in0=MSK[:], in1=FL[:], op=ALU.add)
            BMIN = qpool.tile([128, 32], f32, tag="BMIN")
            nc.vector.tensor_reduce(out=BMIN[:],
                                    in_=NL[:].rearrange("p (c k) -> p c k", k=P),
                                    axis=mybir.AxisListType.X, op=ALU.min)
            # min over heads -> (128, L); clamp to T-1
            BM2 = qpool.tile([128, L], f32, tag="BM2")
            nc.vector.tensor_reduce(out=BM2[:],
                                    in_=_ap(BMIN[:], [[1, L], [4, M]]),
                                    axis=mybir.AxisListType.X, op=ALU.min)
            BASEL = qpool.tile([128, L], f32, tag="BASEL")
            nc.vector.tensor_tensor(out=BASEL[:], in0=BM2[:],
                                    in1=consts[:, C_TM1L:C_TM1L + L], op=ALU.min)

            # gather row indices
            IDXF = qpool.tile([128, L], f32, tag="IDXF")
            nc.vector.tensor_tensor(out=IDXF[:], in0=BASEL[:],
                                    in1=consts[:, C_LST:C_LST + L], op=ALU.add)
            IDX = qpool.tile([128, L], i32, tag="IDX")
            nc.vector.tensor_copy(out=IDX[:], in_=IDXF[:])

            # z = ix - base (all-head base per (q,l))
            Z = qpool.tile([128, 128], f32, tag="Z")
            nc.vector.tensor_tensor(out=Z[:], in0=IX[:],
                                    in1=_ap(BASEL[:], [[0, M], [1, L], [0, P]]),
                                    op=ALU.subtract)

            LSTG = spool.tile([128, 1024], f32, tag="LSTG")
            # levels in groups (0,2), (1,3): per-level ops (ISA: <=3 free dims),
            # one 5D pool-avg per group reduces j for both levels at once
            for grp in ((0, 2), (1, 3)):
                W = WCONF[grp[0]]
                LS = grp[1] - grp[0]
                PRW = M * 16 * DH
                PR = spool.tile([128, 2 * M * 16 * DH], f32, tag="PR")
                G = gpool.tile([128, 2 * W * DM], f32, tag=f"G{grp[0]}")
                for gi, l in enumerate(grp):
                    nf = M * P * W
                    D = spool.tile([128, M * P * 10], f32, tag="D")
                    nc.vector.tensor_tensor(
                        out=D[:, :nf],
                        in0=_ap(Z[:], [[16, M], [1, P], [0, W]], extra_offset=l * P),
                        in1=_ap(consts[:], [[0, M], [0, P], [1, W]],
                                extra_offset=C_JROW),
                        op=ALU.subtract)
                    AB = spool.tile([128, M * P * 10], f32, tag="AB")
                    nc.scalar.activation(out=AB[:, :nf], in_=D[:, :nf], func=ACT.Abs)
                    H = spool.tile([128, M * P * 10], f32, tag="H")
                    nc.scalar.activation(out=H[:, :nf], in_=AB[:, :nf], func=ACT.Relu,
                                         bias=1.0, scale=-1.0)
                    HA = spool.tile([128, M * P * 10], f32, tag="HA")
                    nc.vector.tensor_tensor(
                        out=HA[:, :nf], in0=H[:, :nf],
                        in1=_ap(A[:], [[16, M], [1, P], [0, W]], extra_offset=l * P),
                        op=ALU.mult)
                    U2 = spool.tile([128, M * 2 * 10], f32, tag="U2")
                    nc.vector.tensor_tensor(
                        out=U2[:, :M * 2 * W],
                        in0=_ap(HA[:], [[P * W, M], [W, 2], [1, W]]),
                        in1=_ap(HA[:], [[P * W, M], [W, 2], [1, W]],
                                extra_offset=2 * W),
                        op=ALU.add)
                    U = spool.tile([128, M * 10], f32, tag="U")
                    nc.vector.tensor_tensor(
                        out=U[:, :M * W],
                        in0=_ap(U2[:], [[2 * W, M], [1, W]]),
                        in1=_ap(U2[:], [[2 * W, M], [1, W]], extra_offset=W),
                        op=ALU.add)
                    # gather W full rows per query
                    if os.environ.get("DEFORM_NO_GATHER"):
                        nc.vector.memset(G[:, gi * W * DM:(gi + 1) * W * DM], 0.0)
                    else:
                        nc.gpsimd.indirect_dma_start(
                            out=G[:, gi * W * DM:(gi + 1) * W * DM],
                            out_offset=None,
                            in_=vp[:],
                            in_offset=bass.IndirectOffsetOnAxis(
                                ap=IDX[:, l:l + 1], axis=0),
                            bounds_check=TPR - 1,
                            oob_is_err=False,
                        )
                    # PROD[q, m, j, d] = G[q, j, m, d] * U[q, m, j]
                    muleng = nc.vector if (grp[0] == 0 or os.environ.get('DEFORM_ALL_DVE')) else nc.gpsimd
                    muleng.tensor_tensor(
                        out=_ap(PR[:], [[16 * DH, M], [DH, W], [1, DH]],
                                extra_offset=gi * PRW),
                        in0=_ap(G[:], [[DH, M], [DM, W], [1, DH]],
                                extra_offset=gi * W * DM),
                        in1=_ap(U[:], [[W, M], [1, W], [0, DH]]),
                        op=ALU.mult)
                    # j-tree sum on the other engine; final stage -> LSTG col l
                    eng = nc.vector if os.environ.get('DEFORM_ALL_DVE') else (nc.gpsimd if grp[0] == 0 else nc.vector)
                    w = W
                    while w > 1:
                        h = w // 2
                        last = (h == 1) and (w % 2 == 0)
                        po = gi * PRW
                        dst = (_ap(LSTG[:], [[4, M * DH]], extra_offset=l)
                               if last else
                               _ap(PR[:], [[16 * DH, M], [DH, h], [1, DH]],
                                   extra_offset=po))
                        eng.tensor_tensor(
                            out=dst,
                            in0=_ap(PR[:], [[16 * DH, M], [DH, h], [1, DH]],
                                    extra_offset=po),
                            in1=_ap(PR[:], [[16 * DH, M], [DH, h], [1, DH]],
                                    extra_offset=po + h * DH),
                            op=ALU.add)
                        if w % 2:
                            last2 = h == 1
                            dst2 = (_ap(LSTG[:], [[4, M * DH]], extra_offset=l)
                                    if last2 else
                                    _ap(PR[:], [[16 * DH, M], [1, DH]],
                                        extra_offset=po))
                            eng.tensor_tensor(
                                out=dst2,
                                in0=_ap(PR[:], [[16 * DH, M], [1, DH]],
                                        extra_offset=po),
                                in1=_ap(PR[:], [[16 * DH, M], [1, DH]],
                                        extra_offset=po + (w - 1) * DH),
                                op=ALU.add)
                        w = h

            # sum over levels: LSTG (128, (m,d), 4) -> OUTT (128, 256)
            lse = nc.vector if os.environ.get('DEFORM_ALL_DVE') else nc.gpsimd
            T0 = spool.tile([128, DM], f32, tag="T0")
            lse.tensor_tensor(out=T0[:],
                                    in0=_ap(LSTG[:], [[4, M * DH]]),
                                    in1=_ap(LSTG[:], [[4, M * DH]], extra_offset=1),
                                    op=ALU.add)
            T1 = spool.tile([128, DM], f32, tag="T1")
            lse.tensor_tensor(out=T1[:],
                                    in0=_ap(LSTG[:], [[4, M * DH]], extra_offset=2),
                                    in1=_ap(LSTG[:], [[4, M * DH]], extra_offset=3),
                                    op=ALU.add)
            OUTT = spool.tile([128, DM], f32, tag="OUTT")
            lse.tensor_tensor(out=OUTT[:], in0=T0[:], in1=T1[:], op=ALU.add)
            nc.sync.dma_start(out=out_d[qt * 128:(qt + 1) * 128, :], in_=OUTT[:])

    nc.compile()
    return nc


def host_prep(inputs):
    """Build per-core in_maps from full inputs."""
    q = np.ascontiguousarray(inputs["query"], np.float32)
    ref = np.ascontiguousarray(np.asarray(inputs["reference_points"])[..., 0], np.float32)
    vin = np.ascontiguousarray(inputs["input_flatten"], np.float32)
    W_val = np.asarray(inputs["W_val"], np.float32)
    b_val = np.asarray(inputs["b_val"], np.float32)
    W_off = np.asarray(inputs["W_off"], np.float32)
    b_off = np.asarray(inputs["b_off"], np.float32)
    W_attn = np.asarray(inputs["W_attn"], np.float32)
    b_attn = np.asarray(inputs["b_attn"], np.float32)

    Tvec = np.zeros(M * L * P, np.float32)
    for c in range(M * L * P):
        Tvec[c] = LENS[(c % 16) // 4]
    wv = np.concatenate([W_val.T, b_val[None, :]], 0)
    wof = np.concatenate([(W_off / Tvec[:, None]).T, (b_off / Tvec)[None, :]], 0)
    wat = np.concatenate([W_attn.T, b_attn[None, :]], 0)

    consts = np.zeros((1, CW), np.float32)
    consts[0, C_TVEC:C_TVEC + 128] = Tvec
    for l in range(L):
        consts[0, C_TM1L + l] = LENS[l] - 1
        consts[0, C_LST + l] = LSTARTP[l]
    consts[0, C_JROW:C_JROW + 16] = np.arange(16, dtype=np.float32)
    consts[0, C_NEG1] = -1.0

    shared = {"wv": np.ascontiguousarray(wv), "wof": np.ascontiguousarray(wof),
              "wat": np.ascontiguousarray(wat), "consts": consts}
    return [
        {"q": q[n], "ref": ref[n], "vin": vin[n], **shared}
        for n in range(N)
    ]


_NC_CACHE = None


def kernel(**inputs) -> np.ndarray:
    global _NC_CACHE
    if _NC_CACHE is None:
        _NC_CACHE = build_program()
    nc = _NC_CACHE
    in_maps = host_prep(inputs)
    res = run_bass_kernel_spmd(nc, in_maps, list(range(N)))
    return np.stack([res.results[n]["out"] for n in range(N)]).astype(np.float32)


if __name__ == "__main__":
    d = np.load("/root/problem/cached_io.npz")
    inp = {k: d[k] for k in ["query", "reference_points", "input_flatten",
                             "input_temporal_lens", "input_level_start_index",
                             "W_val", "b_val", "W_off", "b_off", "W_attn", "b_attn"]}
    out = kernel(**inp)
    ref = d["ref_out"]
    err = np.abs(out - ref).max()
    print("absmax err:", err, "scale:", np.abs(ref).max(),
          "rel:", err / np.abs(ref).max())



# revision 12
# speedup vs baseline: 1.5556x; 1.5556x over previous
"""Trainium2 Bass kernel for 1D multi-scale deformable attention (v2).

Self-contained: builds the Bass/Tile program, shards the full inputs
data-parallel over N across 8 NeuronCores, runs via run_bass_kernel_spmd,
and returns the full (N, LQ, 256) output.

v2 vs baseline: bf16 value/gather/weighted-sum datapath (2x DVE modes,
half the gather traffic), raw-offset ix computation (offsets not
pre-divided by T), floor-after-min base computation (casts on (128,4)
instead of (128,128)), paired indirect gathers (2 calls/tile instead
of 4), paired tree reductions, engine-assignment knobs.

Algorithm per core (one batch element):
  value = vin @ W_val.T + b_val            -> padded natural layout (T', 256)
  offs  = q @ W_off.T + b_off              -> ix = ref*T - 0.5 + offs
  attn  = softmax(q @ W_attn.T + b_attn)   per (q, m) over 16 (l,p)
  bilinear + zero padding == sum_t relu(1 - |ix - t|) * V[t], t in [0, T)
  per (q,l): all-head window, base = clamp(floor(min over (m,p) of
  masked relu(ix))), static width W_l; indirect-DMA gathers W_l full
  value rows per query; u[m,j] = sum_p attn * relu(1 - |ix_p - (base+j)|)
  out[q, m*32+d] = sum_{l,j} u * G
"""
import os
import numpy as np
from contextlib import ExitStack

import concourse.bass as bass
import concourse.bacc as bacc
import concourse.tile as tile
from concourse import mybir
from concourse.masks import make_identity
from concourse.bass_utils import run_bass_kernel_spmd

f32 = mybir.dt.float32
bf16 = mybir.dt.bfloat16
i32 = mybir.dt.int32
ALU = mybir.AluOpType
ACT = mybir.ActivationFunctionType

# static problem config
LENS = (2048, 1024, 512, 256)
N, LQ, DM = 8, 2048, 256
M, L, P, DH = 8, 4, 4, 32
S = sum(LENS)                      # 3840
WCONF = (8, 10, 8, 10)             # per-level all-head window rows
PAIRS = ((0, 2), (1, 3))           # levels grouped by equal W
PAD = 12                           # zero rows after each level (>= max(W)-1)
LSTARTP = []
_s = 0
for _T in LENS:
    LSTARTP.append(_s)
    _s += _T + PAD
TPR = _s                           # 3888 padded rows total
NQT = LQ // 128                    # 16 query tiles
NVT = S // 128                     # 30 value tiles
BIG = 100000.0

# consts layout (one row, broadcast to 128 partitions at load)
C_TL = 0             # 4: T_l
C_TM1L = 4           # 4: T_l - 1
C_LST = 8            # 4: LSTARTP[l]
C_JROW = 12          # 16: j = 0..15
C_NEG1 = 28          # 1: -1.0
CW = 29


def _ap(base, dims, extra_offset=0):
    """Custom strided AP derived from a 2D (128, F) contiguous tile AP."""
    return bass.AP(
        tensor=base.tensor,
        offset=base.offset + extra_offset,
        ap=[list(base.ap[0])] + [[s, c] for s, c in dims],
    )


def build_program():
    DT = f32 if os.environ.get("DEFORM_DT") == "f32" else bf16
    prod_gps = os.environ.get("DEFORM_PROD_GPS", "02")     # levels on gpsimd
    tree_gps = os.environ.get("DEFORM_TREE_GPS", "")       # pair ids on gpsimd

    nc = bacc.Bacc("TRN2", target_bir_lowering=False, debug=False)

    q_d = nc.dram_tensor("q", [LQ, DM], f32, kind="ExternalInput")
    ref_d = nc.dram_tensor("ref", [LQ, L], f32, kind="ExternalInput")
    vin_d = nc.dram_tensor("vin", [S, DM], f32, kind="ExternalInput")
    wv_d = nc.dram_tensor("wv", [DM + 1, DM], f32, kind="ExternalInput")
    wof_d = nc.dram_tensor("wof", [DM + 1, M * L * P], f32, kind="ExternalInput")
    wat_d = nc.dram_tensor("wat", [DM + 1, M * L * P], f32, kind="ExternalInput")
    consts_d = nc.dram_tensor("consts", [1, CW], f32, kind="ExternalInput")
    out_d = nc.dram_tensor("out", [LQ, DM], f32, kind="ExternalOutput")

    with tile.TileContext(nc) as tc, ExitStack() as ctx:
        singles = ctx.enter_context(tc.tile_pool(name="singles", bufs=1))
        dram = ctx.enter_context(tc.tile_pool(name="dram", bufs=1, space="DRAM"))
        vpool = ctx.enter_context(tc.tile_pool(name="vpool", bufs=3))
        psum = ctx.enter_context(tc.tile_pool(name="psum", bufs=2, space="PSUM"))
        qpool = ctx.enter_context(tc.tile_pool(name="qpool", bufs=2))
        gpool = ctx.enter_context(tc.tile_pool(name="gpool", bufs=2))
        spool = ctx.enter_context(tc.tile_pool(name="spool", bufs=2))

        # ---- constants / weights (loaded once)
        ident = singles.tile([128, 128], f32)
        make_identity(nc, ident[:])
        ones_row = singles.tile([1, 128], DT)
        nc.vector.memset(ones_row[:], 1.0)
        consts = singles.tile([128, CW], f32)
        nc.sync.dma_start(
            out=consts[:],
            in_=bass.AP(tensor=consts_d[:].tensor, offset=0,
                        ap=[[0, 128], [1, CW]]),
        )
        # load f32 weights then cast once to DT
        wtmp = singles.tile([128, DM], f32)
        wv0 = singles.tile([128, DM], DT)
        wv1 = singles.tile([128, DM], DT)
        wvb = singles.tile([1, DM], DT)
        wof0 = singles.tile([128, 128], DT)
        wof1 = singles.tile([128, 128], DT)
        wofb = singles.tile([1, 128], DT)
        wat0 = singles.tile([128, 128], DT)
        wat1 = singles.tile([128, 128], DT)
        watb = singles.tile([1, 128], DT)
        wtmpb = singles.tile([1, DM], f32)
        for dst, src_d, r0, cols in (
                (wv0, wv_d, 0, DM), (wv1, wv_d, 128, DM),
                (wof0, wof_d, 0, 128), (wof1, wof_d, 128, 128),
                (wat0, wat_d, 0, 128), (wat1, wat_d, 128, 128)):
            nc.sync.dma_start(out=wtmp[:, :cols], in_=src_d[r0:r0 + 128, :])
            nc.scalar.copy(out=dst[:], in_=wtmp[:, :cols])
        for dst, src_d, cols in ((wvb, wv_d, DM), (wofb, wof_d, 128),
                                 (watb, wat_d, 128)):
            nc.sync.dma_start(out=wtmpb[:, :cols], in_=src_d[256:257, :])
            nc.scalar.copy(out=dst[:], in_=wtmpb[:, :cols])

        # ---- value scratch: natural padded rows (TPR, 256) in DT
        vp = dram.tile([TPR, DM], DT)
        zt = singles.tile([128, DM], DT)
        nc.vector.memset(zt[:], 0.0)
        for l, T in enumerate(LENS):
            nc.sync.dma_start(
                out=vp[:][LSTARTP[l] + T:LSTARTP[l] + T + PAD, :],
                in_=zt[:PAD, :])

        # ---- phase A: value projection into vp
        for tt in range(NVT):
            vt = vpool.tile([128, DM], f32, tag="vt")
            nc.sync.dma_start(out=vt[:], in_=vin_d[tt * 128:(tt + 1) * 128, :])
            ps0 = psum.tile([128, 128], f32, tag="tr")
            ps1 = psum.tile([128, 128], f32, tag="tr")
            nc.tensor.transpose(out=ps0[:], in_=vt[:, 0:128], identity=ident[:])
            nc.tensor.transpose(out=ps1[:], in_=vt[:, 128:256], identity=ident[:])
            vT0 = vpool.tile([128, 128], DT, tag="vT")
            vT1 = vpool.tile([128, 128], DT, tag="vT")
            nc.vector.tensor_copy(out=vT0[:], in_=ps0[:])
            nc.scalar.copy(out=vT1[:], in_=ps1[:])
            pv = psum.tile([128, DM], f32, tag="mm")
            nc.tensor.matmul(out=pv[:], lhsT=vT0[:], rhs=wv0[:], start=True, stop=False)
            nc.tensor.matmul(out=pv[:], lhsT=vT1[:], rhs=wv1[:], start=False, stop=False)
            nc.tensor.matmul(out=pv[:], lhsT=ones_row[:], rhs=wvb[:], start=False, stop=True)
            st = vpool.tile([128, DM], DT, tag="st")
            nc.scalar.copy(out=st[:], in_=pv[:])
            row0 = tt * 128
            acc = 0
            for li, T in enumerate(LENS):
                if row0 < acc + T:
                    l, trel = li, row0 - acc
                    break
                acc += T
            dst = LSTARTP[l] + trel
            nc.sync.dma_start(out=vp[:][dst:dst + 128, :], in_=st[:])

        # ---- phase B: per query tile (optionally unrolled repeats for timing)
        rep = max(1, int(os.environ.get("DEFORM_REPEAT", "1")))
        for qt in [i % NQT for i in range(rep * NQT)]:
            qtile = qpool.tile([128, DM], f32, tag="qtile")
            reft = qpool.tile([128, L], f32, tag="reft")
            nc.sync.dma_start(out=qtile[:], in_=q_d[qt * 128:(qt + 1) * 128, :])
            nc.sync.dma_start(out=reft[:], in_=ref_d[qt * 128:(qt + 1) * 128, :])

            psq0 = psum.tile([128, 128], f32, tag="tr")
            psq1 = psum.tile([128, 128], f32, tag="tr")
            nc.tensor.transpose(out=psq0[:], in_=qtile[:, 0:128], identity=ident[:])
            nc.tensor.transpose(out=psq1[:], in_=qtile[:, 128:256], identity=ident[:])
            qT0 = qpool.tile([128, 128], DT, tag="qT")
            qT1 = qpool.tile([128, 128], DT, tag="qT")
            nc.scalar.copy(out=qT0[:], in_=psq0[:])
            nc.scalar.copy(out=qT1[:], in_=psq1[:])

            offp = psum.tile([128, 128], f32, tag="mm")
            nc.tensor.matmul(out=offp[:], lhsT=qT0[:], rhs=wof0[:], start=True, stop=False)
            nc.tensor.matmul(out=offp[:], lhsT=qT1[:], rhs=wof1[:], start=False, stop=False)
            nc.tensor.matmul(out=offp[:], lhsT=ones_row[:], rhs=wofb[:], start=False, stop=True)
            attp = psum.tile([128, 128], f32, tag="mm")
            nc.tensor.matmul(out=attp[:], lhsT=qT0[:], rhs=wat0[:], start=True, stop=False)
            nc.tensor.matmul(out=attp[:], lhsT=qT1[:], rhs=wat1[:], start=False, stop=False)
            nc.tensor.matmul(out=attp[:], lhsT=ones_row[:], rhs=watb[:], start=False, stop=True)

            # softmax numerator (no max-sub: |logits| < ~4); normalization is
            # folded into U via rr
            E = qpool.tile([128, 128], f32, tag="E")
            nc.scalar.activation(out=E[:], in_=attp[:], func=ACT.Exp)
            sm = qpool.tile([128, M], f32, tag="sm")
            nc.vector.tensor_reduce(out=sm[:], in_=E[:].rearrange("p (m k) -> p m k", m=M),
                                    axis=mybir.AxisListType.X, op=ALU.add)
            rr = qpool.tile([128, M], f32, tag="rr")
            nc.vector.reciprocal(out=rr[:], in_=sm[:])

            # ix = ref*T - 0.5 + offs   (offs raw, not pre-divided by T)
            REFTS = qpool.tile([128, L], f32, tag="REFTS")
            nc.vector.tensor_tensor(out=REFTS[:], in0=reft[:],
                                    in1=consts[:, C_TL:C_TL + L], op=ALU.mult)
            nc.vector.tensor_scalar(out=REFTS[:], in0=REFTS[:], scalar1=0.5,
                                    scalar2=None, op0=ALU.subtract)
            IX = qpool.tile([128, 128], f32, tag="IX")
            nc.vector.tensor_tensor(out=IX[:], in0=offp[:],
                                    in1=_ap(REFTS[:], [[0, M], [1, L], [0, P]]),
                                    op=ALU.add)

            # base per (q,l): clamp(floor(min over (m,p) of masked relu(ix)))
            REL = qpool.tile([128, 128], f32, tag="REL")
            nc.scalar.activation(out=REL[:], in_=IX[:], func=ACT.Relu)
            MSK = qpool.tile([128, 128], f32, tag="MSK")
            nc.scalar.activation(out=MSK[:], in_=IX[:], func=ACT.Relu,
                                 bias=consts[:, C_NEG1:C_NEG1 + 1], scale=-1.0)
            MSKs = qpool.tile([128, 128], f32, tag="MSKs")
            nc.vector.tensor_scalar(out=MSKs[:], in0=MSK[:], scalar1=1e13,
                                    scalar2=BIG, op0=ALU.mult, op1=ALU.min)
            NLF = qpool.tile([128, 128], f32, tag="NLF")
            nc.vector.tensor_tensor(out=NLF[:], in0=REL[:], in1=MSKs[:], op=ALU.add)
            BMIN = qpool.tile([128, L], f32, tag="BMIN")
            nc.vector.tensor_reduce(out=BMIN[:],
                                    in_=_ap(NLF[:], [[P, L], [P * L, M], [1, P]]),
                                    axis=mybir.AxisListType.XY, op=ALU.min)
            # floor on the (128, L) mins (floor commutes with min)
            FLI = qpool.tile([128, L], i32, tag="FLI")
            nc.vector.tensor_copy(out=FLI[:], in_=BMIN[:])
            FLR = qpool.tile([128, L], f32, tag="FLR")
            nc.vector.tensor_copy(out=FLR[:], in_=FLI[:])
            GT = qpool.tile([128, L], f32, tag="GT")
            nc.vector.tensor_tensor(out=GT[:], in0=FLR[:], in1=BMIN[:], op=ALU.is_gt)
            FL = qpool.tile([128, L], f32, tag="FL")
            nc.vector.tensor_tensor(out=FL[:], in0=FLR[:], in1=GT[:], op=ALU.subtract)
            BASEL = qpool.tile([128, L], f32, tag="BASEL")
            nc.vector.tensor_tensor(out=BASEL[:], in0=FL[:],
                                    in1=consts[:, C_TM1L:C_TM1L + L], op=ALU.min)
            # IDX columns permuted to pair order (l=0,2,1,3) so each paired
            # gather reads a contiguous 2-column offset AP
            IDXF = qpool.tile([128, L], f32, tag="IDXF")
            nc.vector.tensor_tensor(out=_ap(IDXF[:], [[2, 2], [1, 2]]),
                                    in0=_ap(BASEL[:], [[1, 2], [2, 2]]),
                                    in1=_ap(consts[:], [[1, 2], [2, 2]],
                                            extra_offset=C_LST),
                                    op=ALU.add)
            IDX = qpool.tile([128, L], i32, tag="IDX")
            nc.vector.tensor_copy(out=IDX[:], in_=IDXF[:])

            # z = ix - base (all-head base per (q,l))
            Z = qpool.tile([128, 128], f32, tag="Z")
            nc.vector.tensor_tensor(out=Z[:], in0=IX[:],
                                    in1=_ap(BASEL[:], [[0, M], [1, L], [0, P]]),
                                    op=ALU.subtract)

            # paired gathers: levels (0,2) share W=8, (1,3) share W=10
            GT8 = gpool.tile([128, 2 * 8 * DM], DT, tag="G8")
            GT10 = gpool.tile([128, 2 * 10 * DM], DT, tag="G10")
            # NOTE: multi-index indirect DMA (one call with a (128,2) offset
            # AP) passes CoreSim but returns wrong data on hardware — keep
            # one single-index call per level.
            if not os.environ.get("DEFORM_MERGED_GATHER"):
                for pi, (Gt, W) in enumerate(((GT8, 8), (GT10, 10))):
                    for lv in range(2):
                        nc.gpsimd.indirect_dma_start(
                            out=Gt[:, lv * W * DM:(lv + 1) * W * DM],
                            out_offset=None,
                            in_=vp[:],
                            in_offset=bass.IndirectOffsetOnAxis(
                                ap=IDX[:, 2 * pi + lv:2 * pi + lv + 1], axis=0),
                            bounds_check=TPR - 1,
                            oob_is_err=False,
                        )
            else:
                for pi, (Gt, W) in enumerate(((GT8, 8), (GT10, 10))):
                    nc.gpsimd.indirect_dma_start(
                        out=Gt[:],
                        out_offset=None,
                        in_=vp[:],
                        in_offset=bass.IndirectOffsetOnAxis(
                            ap=IDX[:, 2 * pi:2 * pi + 2], axis=0),
                        bounds_check=TPR - 1,
                        oob_is_err=False,
                    )

            # u[m,j] weights per pair of levels
            HA8 = spool.tile([128, 2 * M * P * 8], f32, tag="HA8")
            HA10 = spool.tile([128, 2 * M * P * 10], f32, tag="HA10")
            D8 = spool.tile([128, 2 * M * P * 8], f32, tag="D8")
            D10 = spool.tile([128, 2 * M * P * 10], f32, tag="D10")
            UB8 = spool.tile([128, 2 * M * 8], DT, tag="UB8")
            UB10 = spool.tile([128, 2 * M * 10], DT, tag="UB10")
            for pi, pair in enumerate(PAIRS):
                W = WCONF[pair[0]]
                Dt = (D8, D10)[pi]
                HAt = (HA8, HA10)[pi]
                UBt = (UB8, UB10)[pi]
                blk = M * P * W
                for lv, l in enumerate(pair):
                    nc.vector.tensor_tensor(
                        out=_ap(Dt[:], [[P * W, M], [W, P], [1, W]],
                                extra_offset=lv * blk),
                        in0=_ap(Z[:], [[2 * M, M], [1, P], [0, W]],
                                extra_offset=l * P),
                        in1=_ap(consts[:], [[0, M], [0, P], [1, W]],
                                extra_offset=C_JROW),
                        op=ALU.subtract)
                # tent = relu(1 - |d|), both levels at once on scalar engine
                nc.scalar.activation(out=Dt[:], in_=Dt[:], func=ACT.Abs)
                nc.scalar.activation(out=Dt[:], in_=Dt[:], func=ACT.Relu,
                                     bias=1.0, scale=-1.0)
                for lv, l in enumerate(pair):
                    nc.vector.tensor_tensor(
                        out=_ap(HAt[:], [[P * W, M], [W, P], [1, W]],
                                extra_offset=lv * blk),
                        in0=_ap(Dt[:], [[P * W, M], [W, P], [1, W]],
                                extra_offset=lv * blk),
                        in1=_ap(E[:], [[2 * M, M], [1, P], [0, W]],
                                extra_offset=l * P),
                        op=ALU.mult)
                # sum over p (both levels at once): (lv,m) fused dim of 16
                U2 = spool.tile([128, 2 * M * 2 * 10], f32, tag=f"U2_{pi}")
                nc.vector.tensor_tensor(
                    out=_ap(U2[:], [[2 * W, 2 * M], [W, 2], [1, W]]),
                    in0=_ap(HAt[:], [[P * W, 2 * M], [2 * W, 2], [1, W]]),
                    in1=_ap(HAt[:], [[P * W, 2 * M], [2 * W, 2], [1, W]],
                            extra_offset=W),
                    op=ALU.add)
                U = spool.tile([128, 2 * M * 10], f32, tag=f"U_{pi}")
                nc.vector.tensor_tensor(
                    out=_ap(U[:], [[W, 2 * M], [1, W]]),
                    in0=_ap(U2[:], [[2 * W, 2 * M], [1, W]]),
                    in1=_ap(U2[:], [[2 * W, 2 * M], [1, W]], extra_offset=W),
                    op=ALU.add)
                # fold softmax normalization into u; cast to DT
                nc.vector.tensor_tensor(
                    out=_ap(UBt[:], [[W, 2 * M], [1, W]]),
                    in0=_ap(U[:], [[W, 2 * M], [1, W]]),
                    in1=_ap(rr[:], [[0, 2], [1, M], [0, W]]),
                    op=ALU.mult)

            # PROD[q, (lv,m), j, d] = G[q, lv, j, (m,d)] * UB[q, (lv,m), j]
            PR8 = spool.tile([128, 2 * M * 8 * DH], DT, tag="PR8")
            PR10 = spool.tile([128, 2 * M * 10 * DH], DT, tag="PR10")
            for pi, pair in enumerate(PAIRS):
                W = WCONF[pair[0]]
                Gt = (GT8, GT10)[pi]
                PRt = (PR8, PR10)[pi]
                UBt = (UB8, UB10)[pi]
                for lv, l in enumerate(pair):
                    eng = nc.gpsimd if str(l) in prod_gps else nc.vector
                    eng.tensor_tensor(
                        out=_ap(PRt[:], [[W * DH, M], [DH, W], [1, DH]],
                                extra_offset=lv * M * W * DH),
                        in0=_ap(Gt[:], [[DH, M], [DM, W], [1, DH]],
                                extra_offset=lv * W * DM),
                        in1=_ap(UBt[:], [[W, M], [1, W], [0, DH]],
                                extra_offset=lv * M * W),
                        op=ALU.mult)

            # j-tree sums per pair, (lv,m) fused dim of 16
            for pi, pair in enumerate(PAIRS):
                W = WCONF[pair[0]]
                PRt = (PR8, PR10)[pi]
                eng = nc.gpsimd if str(pi) in tree_gps else nc.vector
                if W == 10:     # fold j in {8,9} onto {0,1} first
                    eng.tensor_tensor(
                        out=_ap(PRt[:], [[W * DH, 2 * M], [DH, 2], [1, DH]]),
                        in0=_ap(PRt[:], [[W * DH, 2 * M], [DH, 2], [1, DH]]),
                        in1=_ap(PRt[:], [[W * DH, 2 * M], [DH, 2], [1, DH]],
                                extra_offset=8 * DH),
                        op=ALU.add)
                w = 8
                while w > 1:
                    h = w // 2
                    eng.tensor_tensor(
                        out=_ap(PRt[:], [[W * DH, 2 * M], [DH, h], [1, DH]]),
                        in0=_ap(PRt[:], [[W * DH, 2 * M], [DH, h], [1, DH]]),
                        in1=_ap(PRt[:], [[W * DH, 2 * M], [DH, h], [1, DH]],
                                extra_offset=h * DH),
                        op=ALU.add)
                    w = h

            # sum levels: OUT = (PR8_l0 + PR8_l2) + (PR10_l1 + PR10_l3)
            T8 = spool.tile([128, DM], f32, tag="T8")
            nc.vector.tensor_tensor(
                out=T8[:],
                in0=_ap(PR8[:], [[8 * DH, M], [1, DH]]),
                in1=_ap(PR8[:], [[8 * DH, M], [1, DH]], extra_offset=M * 8 * DH),
                op=ALU.add)
            T10 = spool.tile([128, DM], f32, tag="T10")
            nc.vector.tensor_tensor(
                out=T10[:],
                in0=_ap(PR10[:], [[10 * DH, M], [1, DH]]),
                in1=_ap(PR10[:], [[10 * DH, M], [1, DH]], extra_offset=M * 10 * DH),
                op=ALU.add)
            OUTT = spool.tile([128, DM], f32, tag="OUTT")
            nc.vector.tensor_tensor(out=OUTT[:], in0=T8[:], in1=T10[:], op=ALU.add)
            nc.sync.dma_start(out=out_d[qt * 128:(qt + 1) * 128, :], in_=OUTT[:])

    nc.compile()
    return nc


def host_prep(inputs):
    """Build per-core in_maps from full inputs."""
    q = np.ascontiguousarray(inputs["query"], np.float32)
    ref = np.ascontiguousarray(np.asarray(inputs["reference_points"])[..., 0], np.float32)
    vin = np.ascontiguousarray(inputs["input_flatten"], np.float32)
    W_val = np.asarray(inputs["W_val"], np.float32)
    b_val = np.asarray(inputs["b_val"], np.float32)
    W_off = np.asarray(inputs["W_off"], np.float32)
    b_off = np.asarray(inputs["b_off"], np.float32)
    W_attn = np.asarray(inputs["W_attn"], np.float32)
    b_attn = np.asarray(inputs["b_attn"], np.float32)

    wv = np.concatenate([W_val.T, b_val[None, :]], 0)
    wof = np.concatenate([W_off.T, b_off[None, :]], 0)
    wat = np.concatenate([W_attn.T, b_attn[None, :]], 0)

    consts = np.zeros((1, CW), np.float32)
    for l in range(L):
        consts[0, C_TL + l] = LENS[l]
        consts[0, C_TM1L + l] = LENS[l] - 1
        consts[0, C_LST + l] = LSTARTP[l]
    consts[0, C_JROW:C_JROW + 16] = np.arange(16, dtype=np.float32)
    consts[0, C_NEG1] = -1.0

    shared = {"wv": np.ascontiguousarray(wv), "wof": np.ascontiguousarray(wof),
              "wat": np.ascontiguousarray(wat), "consts": consts}
    return [
        {"q": q[n], "ref": ref[n], "vin": vin[n], **shared}
        for n in range(N)
    ]


_NC_CACHE = None


def kernel(**inputs) -> np.ndarray:
    global _NC_CACHE
    if _NC_CACHE is None:
        _NC_CACHE = build_program()
    nc = _NC_CACHE
    in_maps = host_prep(inputs)
    res = run_bass_kernel_spmd(nc, in_maps, list(range(N)))
    return np.stack([res.results[n]["out"] for n in range(N)]).astype(np.float32)


if __name__ == "__main__":
    d = np.load("/root/problem/cached_io.npz")
    inp = {k: d[k] for k in ["query", "reference_points", "input_flatten",
                             "input_temporal_lens", "input_level_start_index",
                             "W_val", "b_val", "W_off", "b_off", "W_attn", "b_attn"]}
    out = kernel(**inp)
    ref = d["ref_out"]
    err = np.abs(out - ref).max()
    print("absmax err:", err, "scale:", np.abs(ref).max(),
          "rel:", err / np.abs(ref).max())


# revision 13
# speedup vs baseline: 1.5662x; 1.0068x over previous
"""Trainium2 Bass kernel for 1D multi-scale deformable attention (v2).

Self-contained: builds the Bass/Tile program, shards the full inputs
data-parallel over N across 8 NeuronCores, runs via run_bass_kernel_spmd,
and returns the full (N, LQ, 256) output.

v2 vs baseline: bf16 value/gather/weighted-sum datapath (2x DVE modes,
half the gather traffic), raw-offset ix computation (offsets not
pre-divided by T), floor-after-min base computation (casts on (128,4)
instead of (128,128)), paired indirect gathers (2 calls/tile instead
of 4), paired tree reductions, engine-assignment knobs.

Algorithm per core (one batch element):
  value = vin @ W_val.T + b_val            -> padded natural layout (T', 256)
  offs  = q @ W_off.T + b_off              -> ix = ref*T - 0.5 + offs
  attn  = softmax(q @ W_attn.T + b_attn)   per (q, m) over 16 (l,p)
  bilinear + zero padding == sum_t relu(1 - |ix - t|) * V[t], t in [0, T)
  per (q,l): all-head window, base = clamp(floor(min over (m,p) of
  masked relu(ix))), static width W_l; indirect-DMA gathers W_l full
  value rows per query; u[m,j] = sum_p attn * relu(1 - |ix_p - (base+j)|)
  out[q, m*32+d] = sum_{l,j} u * G
"""
import os
import numpy as np
from contextlib import ExitStack

import concourse.bass as bass
import concourse.bacc as bacc
import concourse.tile as tile
from concourse import mybir
from concourse.masks import make_identity
from concourse.bass_utils import run_bass_kernel_spmd

f32 = mybir.dt.float32
bf16 = mybir.dt.bfloat16
i32 = mybir.dt.int32
ALU = mybir.AluOpType
ACT = mybir.ActivationFunctionType

# static problem config
LENS = (2048, 1024, 512, 256)
N, LQ, DM = 8, 2048, 256
M, L, P, DH = 8, 4, 4, 32
S = sum(LENS)                      # 3840
WCONF = (8, 10, 8, 10)             # per-level all-head window rows
PAIRS = ((0, 2), (1, 3))           # levels grouped by equal W
PAD = 12                           # zero rows after each level (>= max(W)-1)
LSTARTP = []
_s = 0
for _T in LENS:
    LSTARTP.append(_s)
    _s += _T + PAD
TPR = _s                           # 3888 padded rows total
NQT = LQ // 128                    # 16 query tiles
NVT = S // 128                     # 30 value tiles
BIG = 100000.0

# consts layout (one row, broadcast to 128 partitions at load)
C_TL = 0             # 4: T_l
C_TM1L = 4           # 4: T_l - 1
C_LST = 8            # 4: LSTARTP[l]
C_JROW = 12          # 16: j = 0..15
C_NEG1 = 28          # 1: -1.0
CW = 29


def _ap(base, dims, extra_offset=0):
    """Custom strided AP derived from a 2D (128, F) contiguous tile AP."""
    return bass.AP(
        tensor=base.tensor,
        offset=base.offset + extra_offset,
        ap=[list(base.ap[0])] + [[s, c] for s, c in dims],
    )


def build_program():
    DT = f32 if os.environ.get("DEFORM_DT") == "f32" else bf16
    prod_gps = os.environ.get("DEFORM_PROD_GPS", "02")     # levels on gpsimd
    tree_gps = os.environ.get("DEFORM_TREE_GPS", "")       # pair ids on gpsimd

    nc = bacc.Bacc("TRN2", target_bir_lowering=False, debug=False)

    q_d = nc.dram_tensor("q", [LQ, DM], f32, kind="ExternalInput")
    ref_d = nc.dram_tensor("ref", [LQ, L], f32, kind="ExternalInput")
    vin_d = nc.dram_tensor("vin", [S, DM], f32, kind="ExternalInput")
    wv_d = nc.dram_tensor("wv", [DM + 1, DM], f32, kind="ExternalInput")
    wof_d = nc.dram_tensor("wof", [DM + 1, M * L * P], f32, kind="ExternalInput")
    wat_d = nc.dram_tensor("wat", [DM + 1, M * L * P], f32, kind="ExternalInput")
    consts_d = nc.dram_tensor("consts", [1, CW], f32, kind="ExternalInput")
    out_d = nc.dram_tensor("out", [LQ, DM], f32, kind="ExternalOutput")

    with tile.TileContext(nc) as tc, ExitStack() as ctx:
        singles = ctx.enter_context(tc.tile_pool(name="singles", bufs=1))
        dram = ctx.enter_context(tc.tile_pool(name="dram", bufs=1, space="DRAM"))
        vpool = ctx.enter_context(tc.tile_pool(name="vpool", bufs=3))
        psum = ctx.enter_context(tc.tile_pool(name="psum", bufs=2, space="PSUM"))
        qpool = ctx.enter_context(tc.tile_pool(name="qpool", bufs=2))
        gpool = ctx.enter_context(tc.tile_pool(name="gpool", bufs=2))
        spool = ctx.enter_context(tc.tile_pool(name="spool", bufs=2))

        # ---- constants / weights (loaded once)
        ident = singles.tile([128, 128], f32)
        make_identity(nc, ident[:])
        ones_row = singles.tile([1, 128], DT)
        nc.vector.memset(ones_row[:], 1.0)
        consts = singles.tile([128, CW], f32)
        nc.sync.dma_start(
            out=consts[:],
            in_=bass.AP(tensor=consts_d[:].tensor, offset=0,
                        ap=[[0, 128], [1, CW]]),
        )
        # load f32 weights then cast once to DT
        wtmp = singles.tile([128, DM], f32)
        wv0 = singles.tile([128, DM], DT)
        wv1 = singles.tile([128, DM], DT)
        wvb = singles.tile([1, DM], DT)
        wof0 = singles.tile([128, 128], DT)
        wof1 = singles.tile([128, 128], DT)
        wofb = singles.tile([1, 128], DT)
        wat0 = singles.tile([128, 128], DT)
        wat1 = singles.tile([128, 128], DT)
        watb = singles.tile([1, 128], DT)
        wtmpb = singles.tile([1, DM], f32)
        for dst, src_d, r0, cols in (
                (wv0, wv_d, 0, DM), (wv1, wv_d, 128, DM),
                (wof0, wof_d, 0, 128), (wof1, wof_d, 128, 128),
                (wat0, wat_d, 0, 128), (wat1, wat_d, 128, 128)):
            nc.sync.dma_start(out=wtmp[:, :cols], in_=src_d[r0:r0 + 128, :])
            nc.scalar.copy(out=dst[:], in_=wtmp[:, :cols])
        for dst, src_d, cols in ((wvb, wv_d, DM), (wofb, wof_d, 128),
                                 (watb, wat_d, 128)):
            nc.sync.dma_start(out=wtmpb[:, :cols], in_=src_d[256:257, :])
            nc.scalar.copy(out=dst[:], in_=wtmpb[:, :cols])

        # ---- value scratch: natural padded rows (TPR, 256) in DT
        vp = dram.tile([TPR, DM], DT)
        zt = singles.tile([128, DM], DT)
        nc.vector.memset(zt[:], 0.0)
        for l, T in enumerate(LENS):
            nc.sync.dma_start(
                out=vp[:][LSTARTP[l] + T:LSTARTP[l] + T + PAD, :],
                in_=zt[:PAD, :])

        # ---- phase A: value projection into vp
        for tt in range(NVT):
            vt = vpool.tile([128, DM], f32, tag="vt")
            nc.sync.dma_start(out=vt[:], in_=vin_d[tt * 128:(tt + 1) * 128, :])
            ps0 = psum.tile([128, 128], f32, tag="tr")
            ps1 = psum.tile([128, 128], f32, tag="tr")
            nc.tensor.transpose(out=ps0[:], in_=vt[:, 0:128], identity=ident[:])
            nc.tensor.transpose(out=ps1[:], in_=vt[:, 128:256], identity=ident[:])
            vT0 = vpool.tile([128, 128], DT, tag="vT")
            vT1 = vpool.tile([128, 128], DT, tag="vT")
            nc.vector.tensor_copy(out=vT0[:], in_=ps0[:])
            nc.scalar.copy(out=vT1[:], in_=ps1[:])
            pv = psum.tile([128, DM], f32, tag="mm")
            nc.tensor.matmul(out=pv[:], lhsT=vT0[:], rhs=wv0[:], start=True, stop=False)
            nc.tensor.matmul(out=pv[:], lhsT=vT1[:], rhs=wv1[:], start=False, stop=False)
            nc.tensor.matmul(out=pv[:], lhsT=ones_row[:], rhs=wvb[:], start=False, stop=True)
            st = vpool.tile([128, DM], DT, tag="st")
            nc.scalar.copy(out=st[:], in_=pv[:])
            row0 = tt * 128
            acc = 0
            for li, T in enumerate(LENS):
                if row0 < acc + T:
                    l, trel = li, row0 - acc
                    break
                acc += T
            dst = LSTARTP[l] + trel
            nc.sync.dma_start(out=vp[:][dst:dst + 128, :], in_=st[:])

        # ---- phase B1: per-tile prefix (projections, softmax, base, u
        # weights) for ALL query tiles; overlaps phase A on DVE/Scalar.
        # Results parked in per-tile slots: UB8s/UB10s (u weights) + IDXs.
        UB8s = singles.tile([128, NQT * 2 * M * 8], DT)
        UB10s = singles.tile([128, NQT * 2 * M * 10], DT)
        IDXs = singles.tile([128, NQT * L], i32)

        rep = max(1, int(os.environ.get("DEFORM_REPEAT", "1")))
        for _ in range(rep):
          for qt in range(NQT):
            qtile = qpool.tile([128, DM], f32, tag="qtile")
            reft = qpool.tile([128, L], f32, tag="reft")
            nc.sync.dma_start(out=qtile[:], in_=q_d[qt * 128:(qt + 1) * 128, :])
            nc.sync.dma_start(out=reft[:], in_=ref_d[qt * 128:(qt + 1) * 128, :])

            psq0 = psum.tile([128, 128], f32, tag="tr")
            psq1 = psum.tile([128, 128], f32, tag="tr")
            nc.tensor.transpose(out=psq0[:], in_=qtile[:, 0:128], identity=ident[:])
            nc.tensor.transpose(out=psq1[:], in_=qtile[:, 128:256], identity=ident[:])
            qT0 = qpool.tile([128, 128], DT, tag="qT")
            qT1 = qpool.tile([128, 128], DT, tag="qT")
            nc.scalar.copy(out=qT0[:], in_=psq0[:])
            nc.scalar.copy(out=qT1[:], in_=psq1[:])

            offp = psum.tile([128, 128], f32, tag="mm")
            nc.tensor.matmul(out=offp[:], lhsT=qT0[:], rhs=wof0[:], start=True, stop=False)
            nc.tensor.matmul(out=offp[:], lhsT=qT1[:], rhs=wof1[:], start=False, stop=False)
            nc.tensor.matmul(out=offp[:], lhsT=ones_row[:], rhs=wofb[:], start=False, stop=True)
            attp = psum.tile([128, 128], f32, tag="mm")
            nc.tensor.matmul(out=attp[:], lhsT=qT0[:], rhs=wat0[:], start=True, stop=False)
            nc.tensor.matmul(out=attp[:], lhsT=qT1[:], rhs=wat1[:], start=False, stop=False)
            nc.tensor.matmul(out=attp[:], lhsT=ones_row[:], rhs=watb[:], start=False, stop=True)

            # softmax numerator (no max-sub: |logits| < ~4); normalization is
            # folded into U via rr
            E = qpool.tile([128, 128], f32, tag="E")
            nc.scalar.activation(out=E[:], in_=attp[:], func=ACT.Exp)
            sm = qpool.tile([128, M], f32, tag="sm")
            nc.vector.tensor_reduce(out=sm[:], in_=E[:].rearrange("p (m k) -> p m k", m=M),
                                    axis=mybir.AxisListType.X, op=ALU.add)
            rr = qpool.tile([128, M], f32, tag="rr")
            nc.vector.reciprocal(out=rr[:], in_=sm[:])

            # ix = ref*T - 0.5 + offs   (offs raw, not pre-divided by T)
            REFTS = qpool.tile([128, L], f32, tag="REFTS")
            nc.vector.tensor_tensor(out=REFTS[:], in0=reft[:],
                                    in1=consts[:, C_TL:C_TL + L], op=ALU.mult)
            nc.vector.tensor_scalar(out=REFTS[:], in0=REFTS[:], scalar1=0.5,
                                    scalar2=None, op0=ALU.subtract)
            IX = qpool.tile([128, 128], f32, tag="IX")
            nc.vector.tensor_tensor(out=IX[:], in0=offp[:],
                                    in1=_ap(REFTS[:], [[0, M], [1, L], [0, P]]),
                                    op=ALU.add)

            # base per (q,l): clamp(floor(min over (m,p) of masked relu(ix)))
            REL = qpool.tile([128, 128], f32, tag="REL")
            nc.scalar.activation(out=REL[:], in_=IX[:], func=ACT.Relu)
            MSK = qpool.tile([128, 128], f32, tag="MSK")
            nc.scalar.activation(out=MSK[:], in_=IX[:], func=ACT.Relu,
                                 bias=consts[:, C_NEG1:C_NEG1 + 1], scale=-1.0)
            MSKs = qpool.tile([128, 128], f32, tag="MSKs")
            nc.vector.tensor_scalar(out=MSKs[:], in0=MSK[:], scalar1=1e13,
                                    scalar2=BIG, op0=ALU.mult, op1=ALU.min)
            NLF = qpool.tile([128, 128], f32, tag="NLF")
            nc.vector.tensor_tensor(out=NLF[:], in0=REL[:], in1=MSKs[:], op=ALU.add)
            BMIN = qpool.tile([128, L], f32, tag="BMIN")
            nc.vector.tensor_reduce(out=BMIN[:],
                                    in_=_ap(NLF[:], [[P, L], [P * L, M], [1, P]]),
                                    axis=mybir.AxisListType.XY, op=ALU.min)
            # floor on the (128, L) mins (floor commutes with min)
            FLI = qpool.tile([128, L], i32, tag="FLI")
            nc.vector.tensor_copy(out=FLI[:], in_=BMIN[:])
            FLR = qpool.tile([128, L], f32, tag="FLR")
            nc.vector.tensor_copy(out=FLR[:], in_=FLI[:])
            GT = qpool.tile([128, L], f32, tag="GT")
            nc.vector.tensor_tensor(out=GT[:], in0=FLR[:], in1=BMIN[:], op=ALU.is_gt)
            FL = qpool.tile([128, L], f32, tag="FL")
            nc.vector.tensor_tensor(out=FL[:], in0=FLR[:], in1=GT[:], op=ALU.subtract)
            BASEL = qpool.tile([128, L], f32, tag="BASEL")
            nc.vector.tensor_tensor(out=BASEL[:], in0=FL[:],
                                    in1=consts[:, C_TM1L:C_TM1L + L], op=ALU.min)
            # IDX columns permuted to pair order (l=0,2,1,3); slot qt
            IDXF = qpool.tile([128, L], f32, tag="IDXF")
            nc.vector.tensor_tensor(out=_ap(IDXF[:], [[2, 2], [1, 2]]),
                                    in0=_ap(BASEL[:], [[1, 2], [2, 2]]),
                                    in1=_ap(consts[:], [[1, 2], [2, 2]],
                                            extra_offset=C_LST),
                                    op=ALU.add)
            nc.vector.tensor_copy(out=IDXs[:, qt * L:(qt + 1) * L], in_=IDXF[:])

            # z = ix - base (all-head base per (q,l))
            Z = qpool.tile([128, 128], f32, tag="Z")
            nc.vector.tensor_tensor(out=Z[:], in0=IX[:],
                                    in1=_ap(BASEL[:], [[0, M], [1, L], [0, P]]),
                                    op=ALU.subtract)

            # u[m,j] weights per pair of levels
            D8 = spool.tile([128, 2 * M * P * 8], f32, tag="D8")
            D10 = spool.tile([128, 2 * M * P * 10], f32, tag="D10")
            for pi, pair in enumerate(PAIRS):
                W = WCONF[pair[0]]
                Dt = (D8, D10)[pi]
                UBslot = (UB8s, UB10s)[pi]
                blk = M * P * W
                for lv, l in enumerate(pair):
                    nc.vector.tensor_tensor(
                        out=_ap(Dt[:], [[P * W, M], [W, P], [1, W]],
                                extra_offset=lv * blk),
                        in0=_ap(Z[:], [[2 * M, M], [1, P], [0, W]],
                                extra_offset=l * P),
                        in1=_ap(consts[:], [[0, M], [0, P], [1, W]],
                                extra_offset=C_JROW),
                        op=ALU.subtract)
                # tent = relu(1 - |d|), both levels at once on scalar engine
                nc.scalar.activation(out=Dt[:], in_=Dt[:], func=ACT.Abs)
                nc.scalar.activation(out=Dt[:], in_=Dt[:], func=ACT.Relu,
                                     bias=1.0, scale=-1.0)
                HAt = spool.tile([128, 2 * M * P * 10], f32, tag=f"HA{pi}")
                for lv, l in enumerate(pair):
                    nc.vector.tensor_tensor(
                        out=_ap(HAt[:], [[P * W, M], [W, P], [1, W]],
                                extra_offset=lv * blk),
                        in0=_ap(Dt[:], [[P * W, M], [W, P], [1, W]],
                                extra_offset=lv * blk),
                        in1=_ap(E[:], [[2 * M, M], [1, P], [0, W]],
                                extra_offset=l * P),
                        op=ALU.mult)
                # sum over p (both levels at once): (lv,m) fused dim of 16
                U2 = spool.tile([128, 2 * M * 2 * 10], f32, tag=f"U2_{pi}")
                nc.vector.tensor_tensor(
                    out=_ap(U2[:], [[2 * W, 2 * M], [W, 2], [1, W]]),
                    in0=_ap(HAt[:], [[P * W, 2 * M], [2 * W, 2], [1, W]]),
                    in1=_ap(HAt[:], [[P * W, 2 * M], [2 * W, 2], [1, W]],
                            extra_offset=W),
                    op=ALU.add)
                U = spool.tile([128, 2 * M * 10], f32, tag=f"U_{pi}")
                nc.vector.tensor_tensor(
                    out=_ap(U[:], [[W, 2 * M], [1, W]]),
                    in0=_ap(U2[:], [[2 * W, 2 * M], [1, W]]),
                    in1=_ap(U2[:], [[2 * W, 2 * M], [1, W]], extra_offset=W),
                    op=ALU.add)
                # fold softmax normalization into u; cast to DT; park in slot.
                # Slot layout per tile: [j, lv, m] j-major so PROD's in1
                # iterates (j, m) with j outermost.
                nc.vector.tensor_tensor(
                    out=_ap(UBslot[:], [[1, 2 * M], [2 * M, W]],
                            extra_offset=qt * 2 * M * W),
                    in0=_ap(U[:], [[W, 2 * M], [1, W]]),
                    in1=_ap(rr[:], [[0, 2], [1, M], [0, W]]),
                    op=ALU.mult)

          # ---- phase B2: gather + weighted sum per query tile.
          # Flat j-major layouts: G[j, (m,d)] and PR[lv, j, (m,d)] so the
          # PROD multiplies and j-tree adds are fully contiguous.
          for qt in range(NQT):
            GT8 = gpool.tile([128, 2 * 8 * DM], DT, tag="G8")
            GT10 = gpool.tile([128, 2 * 10 * DM], DT, tag="G10")
            # NOTE: multi-index indirect DMA (one call with a (128,2) offset
            # AP) passes CoreSim but returns wrong data on hardware — keep
            # one single-index call per level.
            for pi, (Gt, W) in enumerate(((GT8, 8), (GT10, 10))):
                for lv in range(2):
                    nc.gpsimd.indirect_dma_start(
                        out=Gt[:, lv * W * DM:(lv + 1) * W * DM],
                        out_offset=None,
                        in_=vp[:],
                        in_offset=bass.IndirectOffsetOnAxis(
                            ap=IDXs[:, qt * L + 2 * pi + lv:
                                    qt * L + 2 * pi + lv + 1], axis=0),
                        oob_is_err=False,
                    )

            # PROD[q, lv, j, (m,d)] = G[q, lv, j, (m,d)] * UB[q, j, lv, m]
            PR8 = spool.tile([128, 2 * 8 * M * DH], DT, tag="PR8")
            PR10 = spool.tile([128, 2 * 10 * M * DH], DT, tag="PR10")
            for pi, pair in enumerate(PAIRS):
                W = WCONF[pair[0]]
                Gt = (GT8, GT10)[pi]
                PRt = (PR8, PR10)[pi]
                UBslot = (UB8s, UB10s)[pi]
                for lv, l in enumerate(pair):
                    eng = nc.gpsimd if str(l) in prod_gps else nc.vector
                    eng.tensor_tensor(
                        out=_ap(PRt[:], [[M * DH, W], [DH, M], [1, DH]],
                                extra_offset=lv * W * M * DH),
                        in0=_ap(Gt[:], [[M * DH, W], [DH, M], [1, DH]],
                                extra_offset=lv * W * DM),
                        in1=_ap(UBslot[:], [[2 * M, W], [1, M], [0, DH]],
                                extra_offset=qt * 2 * M * W + lv * M),
                        op=ALU.mult)

            # j-tree sums per pair: contiguous halves within each lv block
            for pi, pair in enumerate(PAIRS):
                W = WCONF[pair[0]]
                PRt = (PR8, PR10)[pi]
                eng = nc.gpsimd if str(pi) in tree_gps else nc.vector
                if W == 10:     # fold j in {8,9} onto {0,1} first
                    eng.tensor_tensor(
                        out=_ap(PRt[:], [[W * M * DH, 2], [1, 2 * M * DH]]),
                        in0=_ap(PRt[:], [[W * M * DH, 2], [1, 2 * M * DH]]),
                        in1=_ap(PRt[:], [[W * M * DH, 2], [1, 2 * M * DH]],
                                extra_offset=8 * M * DH),
                        op=ALU.add)
                w = 8
                while w > 1:
                    h = w // 2
                    eng.tensor_tensor(
                        out=_ap(PRt[:], [[W * M * DH, 2], [1, h * M * DH]]),
                        in0=_ap(PRt[:], [[W * M * DH, 2], [1, h * M * DH]]),
                        in1=_ap(PRt[:], [[W * M * DH, 2], [1, h * M * DH]],
                                extra_offset=h * M * DH),
                        op=ALU.add)
                    w = h

            # sum levels: OUT = (PR8_l0 + PR8_l2) + (PR10_l1 + PR10_l3)
            T8 = spool.tile([128, DM], f32, tag="T8")
            nc.vector.tensor_tensor(
                out=T8[:], in0=PR8[:, 0:DM],
                in1=_ap(PR8[:], [[1, DM]], extra_offset=8 * M * DH),
                op=ALU.add)
            T10 = spool.tile([128, DM], f32, tag="T10")
            nc.vector.tensor_tensor(
                out=T10[:], in0=PR10[:, 0:DM],
                in1=_ap(PR10[:], [[1, DM]], extra_offset=10 * M * DH),
                op=ALU.add)
            OUTT = spool.tile([128, DM], f32, tag="OUTT")
            nc.vector.tensor_tensor(out=OUTT[:], in0=T8[:], in1=T10[:], op=ALU.add)
            nc.sync.dma_start(out=out_d[qt * 128:(qt + 1) * 128, :], in_=OUTT[:])

    nc.compile()
    return nc


def host_prep(inputs):
    """Build per-core in_maps from full inputs."""
    q = np.ascontiguousarray(inputs["query"], np.float32)
    ref = np.ascontiguousarray(np.asarray(inputs["reference_points"])[..., 0], np.float32)
    vin = np.ascontiguousarray(inputs["input_flatten"], np.float32)
    W_val = np.asarray(inputs["W_val"], np.float32)
    b_val = np.asarray(inputs["b_val"], np.float32)
    W_off = np.asarray(inputs["W_off"], np.float32)
    b_off = np.asarray(inputs["b_off"], np.float32)
    W_attn = np.asarray(inputs["W_attn"], np.float32)
    b_attn = np.asarray(inputs["b_attn"], np.float32)

    wv = np.concatenate([W_val.T, b_val[None, :]], 0)
    wof = np.concatenate([W_off.T, b_off[None, :]], 0)
    wat = np.concatenate([W_attn.T, b_attn[None, :]], 0)

    consts = np.zeros((1, CW), np.float32)
    for l in range(L):
        consts[0, C_TL + l] = LENS[l]
        consts[0, C_TM1L + l] = LENS[l] - 1
        consts[0, C_LST + l] = LSTARTP[l]
    consts[0, C_JROW:C_JROW + 16] = np.arange(16, dtype=np.float32)
    consts[0, C_NEG1] = -1.0

    shared = {"wv": np.ascontiguousarray(wv), "wof": np.ascontiguousarray(wof),
              "wat": np.ascontiguousarray(wat), "consts": consts}
    return [
        {"q": q[n], "ref": ref[n], "vin": vin[n], **shared}
        for n in range(N)
    ]


_NC_CACHE = None


def kernel(**inputs) -> np.ndarray:
    global _NC_CACHE
    if _NC_CACHE is None:
        _NC_CACHE = build_program()
    nc = _NC_CACHE
    in_maps = host_prep(inputs)
    res = run_bass_kernel_spmd(nc, in_maps, list(range(N)))
    return np.stack([res.results[n]["out"] for n in range(N)]).astype(np.float32)


if __name__ == "__main__":
    d = np.load("/root/problem/cached_io.npz")
    inp = {k: d[k] for k in ["query", "reference_points", "input_flatten",
                             "input_temporal_lens", "input_level_start_index",
                             "W_val", "b_val", "W_off", "b_off", "W_attn", "b_attn"]}
    out = kernel(**inp)
    ref = d["ref_out"]
    err = np.abs(out - ref).max()
    print("absmax err:", err, "scale:", np.abs(ref).max(),
          "rel:", err / np.abs(ref).max())
